# revision 1
# baseline (speedup 1.0000x reference)
"""Multi-head attention (B=2, L=4096, C=512, H=8, Dh=64) on 8 trn2 cores.

Sharding: data-parallel over batch (4 cores per batch element) x
tensor-parallel over heads (2 heads per core). Each core computes a partial
output projection; the host sums the 4 partials per batch element and adds
the bias.

Per-core kernel (scores never hit HBM):
  - inputs: xT [512, 4096] (= x[b].T), wq/wk/wv [512, 128] col slices
    (1/sqrt(Dh) folded into wq), wo [128, 512] row slice
  - Q^T, K^T [128, 4096] bf16 via lhsT=W-chunk (f32r), rhs=xT (f32r)
  - V [4096, 128] bf16 stored per head as [V_h(64) | ones(1) | pad(63)]
    so the AV matmul has 128 weight columns (FWL) and produces the softmax
    denominator in output partition 64
  - per q-chunk of 1024, per k-tile, both heads: S^T[k,q] [128,1024] PSUM
    (2 bf16 matmuls each, K=64, head0/head1 at row groups 0/64 so they run
    concurrently on the PE), one ACT exp each -> P^T bf16, then 2 AV bf16
    matmuls each accumulating att^T [128,1024] PSUM over the 32 k-tiles
  - normalize via reciprocal + partition_broadcast + DVE mult -> attn f32r
  - out-proj: out[q,:] = attn.T @ wo, one f32r matmul per 128-row q-tile
"""

import ml_dtypes
import numpy as np

B, L, C, H = 2, 4096, 512, 8
DH = C // H  # 64
P = 128
NCORES = 8
HEADS_PER_CORE = 2
CORES_PER_BATCH = 4

QCHUNK = 1024  # q columns per attention block (2 PSUM banks)
NQC = L // QCHUNK  # 4
NKT = L // P  # 32 k-tiles
NCC = C // P  # 4 contraction chunks for projections

_cached = {}


def _build(reps=1):
    import concourse.mybir as mybir
    import concourse.tile as tile
    from concourse import bacc

    F32R = mybir.dt.float32r
    F32 = mybir.dt.float32
    BF16 = mybir.dt.bfloat16
    EXP = mybir.ActivationFunctionType.Exp
    MULT = mybir.AluOpType.mult

    nc = bacc.Bacc("TRN2", target_bir_lowering=False, debug=False,
                   num_devices=NCORES)
    xT = nc.dram_tensor("xT", [C, L], BF16, kind="ExternalInput").ap()
    wq = nc.dram_tensor("wq", [C, P], BF16, kind="ExternalInput").ap()
    wk = nc.dram_tensor("wk", [C, P], BF16, kind="ExternalInput").ap()
    wv = nc.dram_tensor("wv", [C, P], BF16, kind="ExternalInput").ap()
    wo = nc.dram_tensor("wo", [P, C], BF16, kind="ExternalInput").ap()
    out = nc.dram_tensor("out", [L, C], BF16, kind="ExternalOutput").ap()

    with tile.TileContext(nc) as tc:
        import contextlib
        loop_cm = tc.For_i(0, reps, 1) if reps > 1 else contextlib.nullcontext()
        with (
            tc.tile_pool(name="persist", bufs=1) as persist,
            tc.tile_pool(name="xpool", bufs=1) as xpool,
            tc.tile_pool(name="ptp", bufs=6) as ptp,
            tc.tile_pool(name="small", bufs=2) as small,
            tc.tile_pool(name="outp", bufs=3) as outp,
            loop_cm,
        ):
            # ---- load inputs ----
            wq_t = persist.tile([P, NCC, P], BF16)
            wk_t = persist.tile([P, NCC, P], BF16)
            wv_t = persist.tile([P, NCC, P], BF16)
            wo_t = persist.tile([P, C], BF16)
            nc.sync.dma_start(wq_t, wq.rearrange("(k p) m -> p k m", p=P))
            nc.sync.dma_start(wk_t, wk.rearrange("(k p) m -> p k m", p=P))
            nc.sync.dma_start(wv_t, wv.rearrange("(k p) m -> p k m", p=P))
            nc.sync.dma_start(wo_t, wo)

            xt = xpool.tile([P, NCC, L], BF16)
            xTr = xT.rearrange("(k p) n -> p k n", p=P)
            for j in range(8):  # split the 8.4MB load across DMA queues
                sl = slice(j * (L // 8), (j + 1) * (L // 8))
                nc.sync.dma_start(xt[:, :, sl], xTr[:, :, sl])

            qT = persist.tile([P, L], BF16)
            kT = persist.tile([P, L], BF16)
            # per head block: [V_h (64) | ones (1) | zero pad (63)]
            v_store = persist.tile([P, NKT, HEADS_PER_CORE, P], BF16)
            attn = persist.tile([P, L], BF16)

            # ---- projections ----
            with tc.tile_pool(name="pj_ps", bufs=2, space="PSUM") as pj_ps:
                # Q^T / K^T: [128 (2 heads x 64), L]
                for dst, w_t in ((qT, wq_t), (kT, wk_t)):
                    for j in range(L // 512):
                        ps = pj_ps.tile([P, 512], F32, tag="qk_ps")
                        for c in range(NCC):
                            nc.tensor.matmul(
                                ps, w_t[:, c, :],
                                xt[:, c, j * 512:(j + 1) * 512],
                                start=(c == 0), stop=(c == NCC - 1),
                            )
                        nc.vector.tensor_copy(
                            dst[:, j * 512:(j + 1) * 512], ps)

                # V: per 128-token tile, [tokens, 128] = xT-chunk.T @ wv
                nc.vector.memset(v_store, 0.0)
                ones_t = small.tile([P, NKT], F32, tag="ones")
                nc.vector.memset(ones_t, 1.0)
                for h in range(HEADS_PER_CORE):
                    nc.vector.tensor_copy(v_store[:, :, h, DH], ones_t)
                for r in range(NKT):
                    ps = pj_ps.tile([P, P], F32, tag="v_ps")
                    for c in range(NCC):
                        nc.tensor.matmul(
                            ps, xt[:, c, r * P:(r + 1) * P], wv_t[:, c, :],
                            start=(c == 0), stop=(c == NCC - 1),
                        )
                    for h in range(HEADS_PER_CORE):
                        nc.vector.tensor_copy(
                            v_store[:, r, h, 0:DH],
                            ps[:, h * DH:(h + 1) * DH])

            # ---- attention ----
            s_ps_cm = tc.tile_pool(name="s_ps", bufs=2, space="PSUM")
            a_ps_cm = tc.tile_pool(name="a_ps", bufs=2, space="PSUM")
            s_ps = s_ps_cm.__enter__()
            a_ps = a_ps_cm.__enter__()
            for qc in range(NQC):
                qsl = slice(qc * QCHUNK, (qc + 1) * QCHUNK)
                att = [a_ps.tile([P, QCHUNK], F32, tag="att", name=f"att{_h}")
                       for _h in range(HEADS_PER_CORE)]
                for kt in range(NKT):
                    sps = [s_ps.tile([P, QCHUNK], F32, tag="spsum",
                                    name=f"sps{_h}")
                           for _h in range(HEADS_PER_CORE)]
                    # scores: head0 at PE rows 0:64, head1 at 64:128 (overlap)
                    for h in range(HEADS_PER_CORE):
                        hsl = slice(h * DH, (h + 1) * DH)
                        for j in range(QCHUNK // 512):
                            nc.tensor.matmul(
                                sps[h][:, j * 512:(j + 1) * 512],
                                kT[hsl, kt * P:(kt + 1) * P],
                                qT[hsl, qc * QCHUNK + j * 512:
                                   qc * QCHUNK + (j + 1) * 512],
                                start=True, stop=True,
                            )
                    pts = []
                    for h in range(HEADS_PER_CORE):
                        pt = ptp.tile([P, QCHUNK], BF16, tag="pt")
                        # split PSUM drain between ACT (direct, ~1.75us) and
                        # DVE copy + SBUF-source exp (1.36 + 0.87us): route
                        # 1/3 direct to balance ACT and DVE busy time
                        if (2 * kt + h) % 3 == 0:
                            nc.scalar.activation(pt, sps[h], EXP)
                        else:
                            scp = ptp.tile([P, QCHUNK], F32, tag="scp")
                            nc.vector.tensor_copy(scp, sps[h])
                            nc.scalar.activation(pt, scp, EXP)
                        pts.append(pt)
                    for h in range(HEADS_PER_CORE):
                        for j in range(QCHUNK // 512):
                            nc.tensor.matmul(
                                att[h][:, j * 512:(j + 1) * 512],
                                v_store[:, kt, h, :],
                                pts[h][:, j * 512:(j + 1) * 512],
                                start=(kt == 0), stop=(kt == NKT - 1),
                            )
                # normalize: recip of denominator row, broadcast, multiply
                for h in range(HEADS_PER_CORE):
                    hsl = slice(h * DH, (h + 1) * DH)
                    recip = small.tile([1, QCHUNK], F32, tag="recip")
                    nc.vector.reciprocal(recip, att[h][DH:DH + 1, :])
                    rb = small.tile([DH, QCHUNK], F32, tag="rb")
                    nc.gpsimd.partition_broadcast(rb, recip)
                    nc.vector.tensor_tensor(
                        attn[hsl, qsl], att[h][0:DH, :], rb, MULT)
            a_ps_cm.__exit__(None, None, None)
            s_ps_cm.__exit__(None, None, None)

            # ---- output projection ----
            with tc.tile_pool(name="o_ps", bufs=3, space="PSUM") as o_ps:
                for qt in range(L // P):
                    ps = o_ps.tile([P, C], F32, tag="o_ps")
                    nc.tensor.matmul(ps, attn[:, qt * P:(qt + 1) * P], wo_t,
                                     start=True, stop=True)
                    osb = outp.tile([P, C], BF16, tag="osb")
                    nc.vector.tensor_copy(osb, ps)
                    nc.sync.dma_start(out[qt * P:(qt + 1) * P, :], osb)

    nc.compile()
    return nc


def _get_nc(reps=1):
    key = f"nc{reps}"
    if key not in _cached:
        _cached[key] = _build(reps)
    return _cached[key]


def _build_in_maps(inputs):
    x = np.asarray(inputs["x"], dtype=np.float32)
    Wq = np.asarray(inputs["Wq"], dtype=np.float32)
    Wk = np.asarray(inputs["Wk"], dtype=np.float32)
    Wv = np.asarray(inputs["Wv"], dtype=np.float32)
    Wo = np.asarray(inputs["Wo"], dtype=np.float32)

    scale = np.float32(1.0 / np.sqrt(DH))
    in_maps = []
    for core in range(NCORES):
        b = core // CORES_PER_BATCH
        j = core % CORES_PER_BATCH
        csl = slice(j * P, (j + 1) * P)
        bf = ml_dtypes.bfloat16
        in_maps.append({
            "xT": np.ascontiguousarray(x[b].T.astype(bf)),
            "wq": np.ascontiguousarray((Wq[:, csl] * scale).astype(bf)),
            "wk": np.ascontiguousarray(Wk[:, csl].astype(bf)),
            "wv": np.ascontiguousarray(Wv[:, csl].astype(bf)),
            "wo": np.ascontiguousarray(Wo[csl, :].astype(bf)),
        })
    return in_maps


def kernel(x, Wq, Wk, Wv, Wo, bo):
    from concourse import bass_utils

    bo = np.asarray(bo, dtype=np.float32)
    in_maps = _build_in_maps(
        {"x": x, "Wq": Wq, "Wk": Wk, "Wv": Wv, "Wo": Wo})

    res = bass_utils.run_bass_kernel_spmd(
        _get_nc(), in_maps, core_ids=list(range(NCORES)))

    out = np.zeros((B, L, C), dtype=np.float32)
    for core in range(NCORES):
        out[core // CORES_PER_BATCH] += res.results[core]["out"].astype(np.float32)
    out += bo[None, None, :]
    return out



# revision 2
# speedup vs baseline: 1.0987x; 1.0987x over previous
"""Multi-head attention (B=2, L=4096, C=512, H=8, Dh=64) on 8 trn2 cores.

Sharding: data-parallel over batch (4 cores per batch element) x
tensor-parallel over heads (2 heads per core). Each core computes a partial
output projection; the host sums the 4 partials per batch element and adds
the bias.

Per-core kernel (scores never hit HBM):
  - inputs: xT [512, 4096] (= x[b].T), wq/wk/wv [512, 128] col slices
    (1/sqrt(Dh) folded into wq), wo [128, 512] row slice
  - Q^T, K^T [128, 4096] bf16 via lhsT=W-chunk, rhs=xT
  - V [4096, 128] bf16 stored per head as [V_h(64) | ones(1) | pad(63)]
    so the AV matmul has 128 weight columns and produces the softmax
    denominator in output partition 64
  - per q-chunk of 1024, per k-tile, per head: S^T[k,q] [128,1024] PSUM
    (2 bf16 matmuls, K=64), ONE ACT exp straight from PSUM -> P^T bf16 in
    SBUF (PSUM-direct exp measured ~1.0us/tile, cheaper than the
    DVE-copy+SBUF-exp route), then 2 AV bf16 matmuls accumulating
    att^T [128,1024] PSUM over the 32 k-tiles.
    Heads are staggered (sps ring of 2) and scores for k-tile kt+1 are
    issued before the AV of kt so the PE FIFO never head-of-line blocks
    on an exp.
  - normalize via reciprocal + partition_broadcast + DVE mult -> attn
  - out-proj: out[q,:] = attn.T @ wo, drains split between ACT and DVE
"""

import ml_dtypes
import numpy as np

B, L, C, H = 2, 4096, 512, 8
DH = C // H  # 64
P = 128
NCORES = 8
HEADS_PER_CORE = 2
CORES_PER_BATCH = 4

QCHUNK = 1024  # q columns per attention block (2 PSUM banks)
NQC = L // QCHUNK  # 4
NKT = L // P  # 32 k-tiles
NCC = C // P  # 4 contraction chunks for projections

_cached = {}


def _build(reps=1):
    import concourse.mybir as mybir
    import concourse.tile as tile
    from concourse import bacc

    F32 = mybir.dt.float32
    BF16 = mybir.dt.bfloat16
    EXP = mybir.ActivationFunctionType.Exp
    MULT = mybir.AluOpType.mult

    nc = bacc.Bacc("TRN2", target_bir_lowering=False, debug=False,
                   num_devices=NCORES)
    xT = nc.dram_tensor("xT", [C, L], BF16, kind="ExternalInput").ap()
    wq = nc.dram_tensor("wq", [C, P], BF16, kind="ExternalInput").ap()
    wk = nc.dram_tensor("wk", [C, P], BF16, kind="ExternalInput").ap()
    wv = nc.dram_tensor("wv", [C, P], BF16, kind="ExternalInput").ap()
    wo = nc.dram_tensor("wo", [P, C], BF16, kind="ExternalInput").ap()
    out = nc.dram_tensor("out", [L, C], BF16, kind="ExternalOutput").ap()

    with tile.TileContext(nc) as tc:
        import contextlib
        loop_cm = tc.For_i(0, reps, 1) if reps > 1 else contextlib.nullcontext()
        with (
            tc.tile_pool(name="persist", bufs=1) as persist,
            tc.tile_pool(name="xpool", bufs=1) as xpool,
            tc.tile_pool(name="ptp", bufs=6) as ptp,
            tc.tile_pool(name="small", bufs=2) as small,
            tc.tile_pool(name="outp", bufs=4) as outp,
            loop_cm,
        ):
            # preload the exp table set so the first real exp doesn't pay
            # the ~2.7us ACT_TABLE_LOAD mid-pipeline
            warm_in = small.tile([1, 8], F32, tag="warm_in")
            warm_out = small.tile([1, 8], F32, tag="warm_out")
            nc.vector.memset(warm_in, 0.0)
            nc.scalar.activation(warm_out, warm_in, EXP)

            # ---- load inputs ----
            wq_t = persist.tile([P, NCC, P], BF16)
            wk_t = persist.tile([P, NCC, P], BF16)
            wv_t = persist.tile([P, NCC, P], BF16)
            wo_t = persist.tile([P, C], BF16)
            nc.sync.dma_start(wq_t, wq.rearrange("(k p) m -> p k m", p=P))
            nc.sync.dma_start(wk_t, wk.rearrange("(k p) m -> p k m", p=P))
            nc.sync.dma_start(wv_t, wv.rearrange("(k p) m -> p k m", p=P))
            nc.sync.dma_start(wo_t, wo)

            xt = xpool.tile([P, NCC, L], BF16)
            xTr = xT.rearrange("(k p) n -> p k n", p=P)
            for j in range(8):  # split the 8.4MB load across DMA queues
                sl = slice(j * (L // 8), (j + 1) * (L // 8))
                nc.sync.dma_start(xt[:, :, sl], xTr[:, :, sl])

            qT = persist.tile([P, L], BF16)
            kT = persist.tile([P, L], BF16)
            # per head block: [V_h (64) | ones (1) | zero pad (63)]
            v_store = persist.tile([P, NKT, HEADS_PER_CORE, P], BF16)
            attn = persist.tile([P, L], BF16)

            # ---- projections ----
            with tc.tile_pool(name="pj_ps", bufs=2, space="PSUM") as pj_ps:
                # Q^T / K^T: [128 (2 heads x 64), L]; drains alternate
                # between ACT and DVE so neither engine serializes the phase
                drain_idx = 0
                for dst, w_t in ((kT, wk_t), (qT, wq_t)):
                    for j in range(L // 512):
                        ps = pj_ps.tile([P, 512], F32, tag="qk_ps")
                        for c in range(NCC):
                            nc.tensor.matmul(
                                ps, w_t[:, c, :],
                                xt[:, c, j * 512:(j + 1) * 512],
                                start=(c == 0), stop=(c == NCC - 1),
                            )
                        dsl = dst[:, j * 512:(j + 1) * 512]
                        if drain_idx % 2 == 0:
                            nc.scalar.copy(dsl, ps)
                        else:
                            nc.vector.tensor_copy(dsl, ps)
                        drain_idx += 1

                # V: per 128-token tile, [tokens, 128] = xT-chunk.T @ wv
                nc.vector.memset(v_store, 0.0)
                ones_t = small.tile([P, NKT], F32, tag="ones")
                nc.vector.memset(ones_t, 1.0)
                for h in range(HEADS_PER_CORE):
                    nc.vector.tensor_copy(v_store[:, :, h, DH], ones_t)
                for r in range(NKT):
                    ps = pj_ps.tile([P, P], F32, tag="v_ps")
                    for c in range(NCC):
                        nc.tensor.matmul(
                            ps, xt[:, c, r * P:(r + 1) * P], wv_t[:, c, :],
                            start=(c == 0), stop=(c == NCC - 1),
                        )
                    nc.vector.tensor_copy(v_store[:, r, 0, 0:DH], ps[:, 0:DH])
                    nc.scalar.copy(v_store[:, r, 1, 0:DH], ps[:, DH:2 * DH])

            # ---- attention ----
            # Software-pipelined by one k-tile: issue scores(kt) for both
            # heads, then exp(kt-1) consumers' AV. sps ring of 2 means
            # scores(kt+1,h) waits only on exp(kt,h) having drained.
            s_ps_cm = tc.tile_pool(name="s_ps", bufs=2, space="PSUM")
            a_ps_cm = tc.tile_pool(name="a_ps", bufs=2, space="PSUM")
            s_ps = s_ps_cm.__enter__()
            a_ps = a_ps_cm.__enter__()
            for qc in range(NQC):
                qsl = slice(qc * QCHUNK, (qc + 1) * QCHUNK)
                att = [a_ps.tile([P, QCHUNK], F32, tag="att", name=f"att{_h}")
                       for _h in range(HEADS_PER_CORE)]
                pend = []  # (h, pt) waiting for their AV matmuls
                for kt in range(NKT):
                    for h in range(HEADS_PER_CORE):
                        hsl = slice(h * DH, (h + 1) * DH)
                        sps = s_ps.tile([P, QCHUNK], F32, tag="spsum",
                                        name=f"sps{h}")
                        for j in range(QCHUNK // 512):
                            nc.tensor.matmul(
                                sps[:, j * 512:(j + 1) * 512],
                                kT[hsl, kt * P:(kt + 1) * P],
                                qT[hsl, qc * QCHUNK + j * 512:
                                   qc * QCHUNK + (j + 1) * 512],
                                start=True, stop=True,
                            )
                        pt = ptp.tile([P, QCHUNK], BF16, tag="pt")
                        nc.scalar.activation(pt, sps, EXP)
                        pend.append((kt, h, pt))
                        # drain AV one (kt,h) behind so the PE never
                        # head-of-line blocks waiting for the exp
                        if len(pend) > 1:
                            okt, oh, opt = pend.pop(0)
                            for j in range(QCHUNK // 512):
                                nc.tensor.matmul(
                                    att[oh][:, j * 512:(j + 1) * 512],
                                    v_store[:, okt, oh, :],
                                    opt[:, j * 512:(j + 1) * 512],
                                    start=(okt == 0), stop=(okt == NKT - 1),
                                )
                for okt, oh, opt in pend:
                    for j in range(QCHUNK // 512):
                        nc.tensor.matmul(
                            att[oh][:, j * 512:(j + 1) * 512],
                            v_store[:, okt, oh, :],
                            opt[:, j * 512:(j + 1) * 512],
                            start=(okt == 0), stop=(okt == NKT - 1),
                        )
                # normalize: recip of denominator row, broadcast, multiply
                for h in range(HEADS_PER_CORE):
                    hsl = slice(h * DH, (h + 1) * DH)
                    recip = small.tile([1, QCHUNK], F32, tag="recip")
                    nc.vector.reciprocal(recip, att[h][DH:DH + 1, :])
                    rb = small.tile([DH, QCHUNK], F32, tag="rb")
                    nc.gpsimd.partition_broadcast(rb, recip)
                    nc.vector.tensor_tensor(
                        attn[hsl, qsl], att[h][0:DH, :], rb, MULT)
            a_ps_cm.__exit__(None, None, None)
            s_ps_cm.__exit__(None, None, None)

            # ---- output projection ----
            with tc.tile_pool(name="o_ps", bufs=3, space="PSUM") as o_ps:
                for qt in range(L // P):
                    ps = o_ps.tile([P, C], F32, tag="o_ps")
                    nc.tensor.matmul(ps, attn[:, qt * P:(qt + 1) * P], wo_t,
                                     start=True, stop=True)
                    osb = outp.tile([P, C], BF16, tag="osb")
                    if qt % 2 == 0:
                        nc.scalar.copy(osb, ps)
                    else:
                        nc.vector.tensor_copy(osb, ps)
                    nc.sync.dma_start(out[qt * P:(qt + 1) * P, :], osb)

    nc.compile()
    return nc


def _get_nc(reps=1):
    key = f"nc{reps}"
    if key not in _cached:
        _cached[key] = _build(reps)
    return _cached[key]


def _build_in_maps(inputs):
    x = np.asarray(inputs["x"], dtype=np.float32)
    Wq = np.asarray(inputs["Wq"], dtype=np.float32)
    Wk = np.asarray(inputs["Wk"], dtype=np.float32)
    Wv = np.asarray(inputs["Wv"], dtype=np.float32)
    Wo = np.asarray(inputs["Wo"], dtype=np.float32)

    scale = np.float32(1.0 / np.sqrt(DH))
    in_maps = []
    for core in range(NCORES):
        b = core // CORES_PER_BATCH
        j = core % CORES_PER_BATCH
        csl = slice(j * P, (j + 1) * P)
        bf = ml_dtypes.bfloat16
        in_maps.append({
            "xT": np.ascontiguousarray(x[b].T.astype(bf)),
            "wq": np.ascontiguousarray((Wq[:, csl] * scale).astype(bf)),
            "wk": np.ascontiguousarray(Wk[:, csl].astype(bf)),
            "wv": np.ascontiguousarray(Wv[:, csl].astype(bf)),
            "wo": np.ascontiguousarray(Wo[csl, :].astype(bf)),
        })
    return in_maps


def kernel(x, Wq, Wk, Wv, Wo, bo):
    from concourse import bass_utils

    bo = np.asarray(bo, dtype=np.float32)
    in_maps = _build_in_maps(
        {"x": x, "Wq": Wq, "Wk": Wk, "Wv": Wv, "Wo": Wo})

    res = bass_utils.run_bass_kernel_spmd(
        _get_nc(), in_maps, core_ids=list(range(NCORES)))

    out = np.zeros((B, L, C), dtype=np.float32)
    for core in range(NCORES):
        out[core // CORES_PER_BATCH] += res.results[core]["out"].astype(np.float32)
    out += bo[None, None, :]
    return out


# revision 5
# speedup vs baseline: 1.7996x; 1.6379x over previous
"""Multi-head attention (B=2, L=4096, C=512, H=8, Dh=64) on 8 trn2 cores.

Sharding: data-parallel over batch (4 cores per batch element) x
tensor-parallel over heads (2 heads per core). Each core computes per-head
UNNORMALIZED partial outputs plus softmax denominators; the host divides by
the denominators, sums the partials, and adds the bias.

Per-core kernel (scores never hit HBM):
  - inputs: xT [512, 4096] (= x[b].T), wq/wk/wv [512, 128] col slices
    (1/sqrt(Dh) folded into wq), wo [128, 512] row slice
  - Q^T, K^T [128, 4096] bf16 via lhsT=W-chunk, rhs=xT
  - V [4096, 128] bf16 stored per head as [V_h(64) | ones(1) | pad(63)]
    so the AV matmul produces the softmax denominator in output row 64
  - attention in q-chunks of 512: per k-tile both heads' scores go into one
    [128, 1024] PSUM super-tile (2 banks, ring of 3). The exp alternates
    between engines per k-tile:
      even kt: ScalarE activation Exp (PSUM-direct -> bf16 SBUF)
      odd  kt: VectorE Schraudolph bit-trick exp: y_bits = int16(x*A + B)
               viewed as bf16 (exp2 linear-in-mantissa approximation; the
               softmax denominator cancels its common-mode error; end-to-end
               rel err ~6.5e-3 in simulation)
    This gives the PE two independent exp servers so the attention loop is
    PE-bound (the PE stays busy -> HAM keeps the 2.4GHz clock).
  - AV: per (kt, h) one bf16 matmul accumulating att^T [128,512] PSUM
    (1 bank per head) over the 32 k-tiles, lagging the scores by one k-tile
    so the PE FIFO never blocks on an exp.
  - att rows [0:65] (values + denominator) drain once per (qc, h) into
    per-head attn tiles; out-proj per head: out_h[q,:] = attn_h.T @ wo_h.
"""

import ml_dtypes
import numpy as np

B, L, C, H = 2, 4096, 512, 8
DH = C // H  # 64
P = 128
NCORES = 8
HEADS_PER_CORE = 2
CORES_PER_BATCH = 4

QCHUNK = 512  # q columns per attention block (1 PSUM bank per head)
NQC = L // QCHUNK  # 8
NKT = L // P  # 32 k-tiles
NCC = C // P  # 4 contraction chunks for projections

# Schraudolph bf16 exp: bits = int16(x * SCH_A + SCH_B); view as bf16
SCH_A = 128 * 1.4426950408889634  # 128 * log2(e)
SCH_B = 16248.5

_cached = {}


def _build(reps=1):
    import concourse.mybir as mybir
    import concourse.tile as tile
    from concourse import bacc

    F32 = mybir.dt.float32
    BF16 = mybir.dt.bfloat16
    I16 = mybir.dt.int16
    EXP = mybir.ActivationFunctionType.Exp
    MULT = mybir.AluOpType.mult
    ADD = mybir.AluOpType.add

    nc = bacc.Bacc("TRN2", target_bir_lowering=False, debug=False,
                   num_devices=NCORES)
    xT = nc.dram_tensor("xT", [C, L], BF16, kind="ExternalInput").ap()
    wq = nc.dram_tensor("wq", [C, P], BF16, kind="ExternalInput").ap()
    wk = nc.dram_tensor("wk", [C, P], BF16, kind="ExternalInput").ap()
    wv = nc.dram_tensor("wv", [C, P], BF16, kind="ExternalInput").ap()
    wo = nc.dram_tensor("wo", [P, C], BF16, kind="ExternalInput").ap()
    out0 = nc.dram_tensor("out0", [L, C], BF16, kind="ExternalOutput").ap()
    out1 = nc.dram_tensor("out1", [L, C], BF16, kind="ExternalOutput").ap()
    den01 = nc.dram_tensor("den01", [HEADS_PER_CORE, L], BF16,
                           kind="ExternalOutput").ap()

    with tile.TileContext(nc) as tc:
        import contextlib
        loop_cm = tc.For_i(0, reps, 1) if reps > 1 else contextlib.nullcontext()
        with (
            tc.tile_pool(name="persist", bufs=1) as persist,
            tc.tile_pool(name="xpool", bufs=1) as xpool,
            tc.tile_pool(name="ptp", bufs=5) as ptp,
            tc.tile_pool(name="small", bufs=2) as small,
            tc.tile_pool(name="outp", bufs=4) as outp,
            loop_cm,
        ):
            # preload the exp table set so the first real exp doesn't pay
            # the ~2.7us ACT_TABLE_LOAD mid-pipeline
            warm_in = small.tile([1, 8], F32, tag="warm_in")
            warm_out = small.tile([1, 8], F32, tag="warm_out")
            nc.vector.memset(warm_in, 0.0)
            nc.scalar.activation(warm_out, warm_in, EXP)

            # ---- load inputs ----
            wq_t = persist.tile([P, NCC, P], BF16)
            wk_t = persist.tile([P, NCC, P], BF16)
            wv_t = persist.tile([P, NCC, P], BF16)
            # wo rows per head, both at base partition 0 (matmul operands
            # must share base_partition with the attn lhsT)
            wo_th = [persist.tile([DH, C], BF16, name=f"wo_t{_h}")
                     for _h in range(HEADS_PER_CORE)]
            nc.sync.dma_start(wq_t, wq.rearrange("(k p) m -> p k m", p=P))
            nc.sync.dma_start(wk_t, wk.rearrange("(k p) m -> p k m", p=P))
            nc.sync.dma_start(wv_t, wv.rearrange("(k p) m -> p k m", p=P))
            for _h in range(HEADS_PER_CORE):
                nc.sync.dma_start(wo_th[_h], wo[_h * DH:(_h + 1) * DH, :])

            xt = xpool.tile([P, NCC, L], BF16)
            xTr = xT.rearrange("(k p) n -> p k n", p=P)
            for j in range(8):  # split the 8.4MB load across DMA queues
                sl = slice(j * (L // 8), (j + 1) * (L // 8))
                nc.sync.dma_start(xt[:, :, sl], xTr[:, :, sl])

            qT = persist.tile([P, L], BF16)
            kT = persist.tile([P, L], BF16)
            # per head block: [V_h (64) | ones (1) | zero pad (63)]
            v_store = persist.tile([P, NKT, HEADS_PER_CORE, P], BF16)
            # rows 0:64 = unnormalized attention, row 64 = denominator
            attn = [persist.tile([DH + 1, L], BF16, name=f"attn{_h}")
                    for _h in range(HEADS_PER_CORE)]

            # ---- projections ----
            with tc.tile_pool(name="pj_ps", bufs=2, space="PSUM") as pj_ps:
                drain_idx = 0
                for dst, w_t in ((kT, wk_t), (qT, wq_t)):
                    for j in range(L // 512):
                        ps = pj_ps.tile([P, 512], F32, tag="qk_ps")
                        for c in range(NCC):
                            nc.tensor.matmul(
                                ps, w_t[:, c, :],
                                xt[:, c, j * 512:(j + 1) * 512],
                                start=(c == 0), stop=(c == NCC - 1),
                            )
                        dsl = dst[:, j * 512:(j + 1) * 512]
                        if drain_idx % 2 == 0:
                            nc.scalar.copy(dsl, ps)
                        else:
                            nc.vector.tensor_copy(dsl, ps)
                        drain_idx += 1

                # V: per 128-token tile, [tokens, 128] = xT-chunk.T @ wv
                nc.vector.memset(v_store, 0.0)
                ones_t = small.tile([P, NKT], F32, tag="ones")
                nc.vector.memset(ones_t, 1.0)
                for h in range(HEADS_PER_CORE):
                    nc.vector.tensor_copy(v_store[:, :, h, DH], ones_t)
                for r in range(NKT):
                    ps = pj_ps.tile([P, P], F32, tag="v_ps")
                    for c in range(NCC):
                        nc.tensor.matmul(
                            ps, xt[:, c, r * P:(r + 1) * P], wv_t[:, c, :],
                            start=(c == 0), stop=(c == NCC - 1),
                        )
                    nc.vector.tensor_copy(v_store[:, r, 0, 0:DH], ps[:, 0:DH])
                    nc.scalar.copy(v_store[:, r, 1, 0:DH], ps[:, DH:2 * DH])

            # ---- attention ----
            s_ps_cm = tc.tile_pool(name="s_ps", bufs=3, space="PSUM")
            a_ps_cm = tc.tile_pool(name="a_ps", bufs=2, space="PSUM")
            s_ps = s_ps_cm.__enter__()
            a_ps = a_ps_cm.__enter__()
            for qc in range(NQC):
                qsl = slice(qc * QCHUNK, (qc + 1) * QCHUNK)
                att = [a_ps.tile([P, QCHUNK], F32, tag="att", name=f"att{_h}")
                       for _h in range(HEADS_PER_CORE)]
                pend = []  # (kt, pt) waiting for their AV matmuls
                for kt in range(NKT):
                    sps = s_ps.tile([P, 2 * QCHUNK], F32, tag="spsum")
                    for h in range(HEADS_PER_CORE):
                        hsl = slice(h * DH, (h + 1) * DH)
                        nc.tensor.matmul(
                            sps[:, h * QCHUNK:(h + 1) * QCHUNK],
                            kT[hsl, kt * P:(kt + 1) * P],
                            qT[hsl, qsl],
                            start=True, stop=True,
                        )
                    pt = ptp.tile([P, 2 * QCHUNK], BF16, tag="pt")
                    if kt % 2 == 1:
                        nc.vector.tensor_scalar(
                            pt.bitcast(I16), sps, SCH_A, SCH_B, MULT, ADD)
                    else:
                        nc.scalar.activation(pt, sps, EXP)
                    pend.append((kt, pt))
                    if len(pend) > 1:
                        okt, opt = pend.pop(0)
                        for h in range(HEADS_PER_CORE):
                            nc.tensor.matmul(
                                att[h],
                                v_store[:, okt, h, :],
                                opt[:, h * QCHUNK:(h + 1) * QCHUNK],
                                start=(okt == 0), stop=(okt == NKT - 1),
                            )
                for okt, opt in pend:
                    for h in range(HEADS_PER_CORE):
                        nc.tensor.matmul(
                            att[h],
                            v_store[:, okt, h, :],
                            opt[:, h * QCHUNK:(h + 1) * QCHUNK],
                            start=(okt == 0), stop=(okt == NKT - 1),
                        )
                # drain unnormalized attention + denominator row
                nc.scalar.copy(attn[0][:, qsl], att[0][0:DH + 1, :])
                nc.vector.tensor_copy(attn[1][:, qsl], att[1][0:DH + 1, :])
            a_ps_cm.__exit__(None, None, None)
            s_ps_cm.__exit__(None, None, None)

            # ---- output projection (per head, unnormalized) ----
            with tc.tile_pool(name="o_ps", bufs=4, space="PSUM") as o_ps:
                for qt in range(L // P):
                    for h, out_h in ((0, out0), (1, out1)):
                        ps = o_ps.tile([P, C], F32, tag="o_ps")
                        nc.tensor.matmul(
                            ps, attn[h][0:DH, qt * P:(qt + 1) * P],
                            wo_th[h], start=True, stop=True)
                        osb = outp.tile([P, C], BF16, tag="osb")
                        if (2 * qt + h) % 2 == 0:
                            nc.scalar.copy(osb, ps)
                        else:
                            nc.vector.tensor_copy(osb, ps)
                        nc.sync.dma_start(out_h[qt * P:(qt + 1) * P, :], osb)
                for h in range(HEADS_PER_CORE):
                    nc.sync.dma_start(den01[h:h + 1, :], attn[h][DH:DH + 1, :])

    nc.compile()
    return nc


def _get_nc(reps=1):
    key = f"nc{reps}"
    if key not in _cached:
        _cached[key] = _build(reps)
    return _cached[key]


def _build_in_maps(inputs):
    x = np.asarray(inputs["x"], dtype=np.float32)
    Wq = np.asarray(inputs["Wq"], dtype=np.float32)
    Wk = np.asarray(inputs["Wk"], dtype=np.float32)
    Wv = np.asarray(inputs["Wv"], dtype=np.float32)
    Wo = np.asarray(inputs["Wo"], dtype=np.float32)

    scale = np.float32(1.0 / np.sqrt(DH))
    in_maps = []
    for core in range(NCORES):
        b = core // CORES_PER_BATCH
        j = core % CORES_PER_BATCH
        csl = slice(j * P, (j + 1) * P)
        bf = ml_dtypes.bfloat16
        in_maps.append({
            "xT": np.ascontiguousarray(x[b].T.astype(bf)),
            "wq": np.ascontiguousarray((Wq[:, csl] * scale).astype(bf)),
            "wk": np.ascontiguousarray(Wk[:, csl].astype(bf)),
            "wv": np.ascontiguousarray(Wv[:, csl].astype(bf)),
            "wo": np.ascontiguousarray(Wo[csl, :].astype(bf)),
        })
    return in_maps


def kernel(x, Wq, Wk, Wv, Wo, bo):
    from concourse import bass_utils

    bo = np.asarray(bo, dtype=np.float32)
    in_maps = _build_in_maps(
        {"x": x, "Wq": Wq, "Wk": Wk, "Wv": Wv, "Wo": Wo})

    res = bass_utils.run_bass_kernel_spmd(
        _get_nc(), in_maps, core_ids=list(range(NCORES)))

    out = np.zeros((B, L, C), dtype=np.float32)
    for core in range(NCORES):
        r = res.results[core]
        den = np.asarray(r["den01"]).astype(np.float32)  # [2, L]
        b = core // CORES_PER_BATCH
        out[b] += np.asarray(r["out0"]).astype(np.float32) / den[0][:, None]
        out[b] += np.asarray(r["out1"]).astype(np.float32) / den[1][:, None]
    out += bo[None, None, :]
    return out


# revision 7
# speedup vs baseline: 3.0041x; 1.6693x over previous
"""Multi-head attention (B=2, L=4096, C=512, H=8, Dh=64) on 8 trn2 cores.

Sharding: data-parallel over batch (4 cores per batch element) x
tensor-parallel over heads (2 heads per core). Each core computes per-head
UNNORMALIZED partial outputs plus softmax denominators; the host divides by
the denominators, sums the partials, and adds the bias.

Per-core kernel (scores never hit HBM):
  - inputs: xT [512, 4096] (= x[b].T), wq/wk/wv [512, 128] col slices
    (1/sqrt(Dh) folded into wq), wo [128, 512] row slice
  - Q^T, K^T [128, 4096] bf16 via lhsT=W-chunk, rhs=xT
  - V [4096, 128] bf16 stored per head as [V_h(64) | ones(1) | pad(63)]
    so the AV matmul produces the softmax denominator in output row 64
  - attention in q-chunks of 512: per k-tile both heads' scores go into one
    [128, 1024] PSUM super-tile (2 banks, ring of 3). The exp alternates
    between engines per k-tile:
      even kt: ScalarE activation Exp (PSUM-direct -> bf16 SBUF)
      odd  kt: VectorE Schraudolph bit-trick exp: y_bits = int16(x*A + B)
               viewed as bf16 (exp2 linear-in-mantissa approximation; the
               softmax denominator cancels its common-mode error; end-to-end
               rel err ~6.5e-3 in simulation)
    This gives the PE two independent exp servers so the attention loop is
    PE-bound (the PE stays busy -> HAM keeps the 2.4GHz clock).
  - AV: per (kt, h) one bf16 matmul accumulating att^T [128,512] PSUM
    (1 bank per head) over the 32 k-tiles, lagging the scores by one k-tile
    so the PE FIFO never blocks on an exp.
  - att rows [0:65] (values + denominator) drain once per (qc, h) into
    per-head attn tiles; out-proj per head: out_h[q,:] = attn_h.T @ wo_h.
"""

import ml_dtypes
import numpy as np

B, L, C, H = 2, 4096, 512, 8
DH = C // H  # 64
P = 128
NCORES = 8
HEADS_PER_CORE = 2
CORES_PER_BATCH = 4

QCHUNK = 512  # q columns per attention block (1 PSUM bank per head)
NQC = L // QCHUNK  # 8
NKT = L // P  # 32 k-tiles
NCC = C // P  # 4 contraction chunks for projections

# Schraudolph bf16 exp: bits = int16(x * SCH_A + SCH_B); view as bf16
SCH_A = 128 * 1.4426950408889634  # 128 * log2(e)
SCH_B = 16248.5

_cached = {}


def _build(reps=1):
    import concourse.mybir as mybir
    import concourse.tile as tile
    from concourse import bacc

    F32 = mybir.dt.float32
    BF16 = mybir.dt.bfloat16
    I16 = mybir.dt.int16
    EXP = mybir.ActivationFunctionType.Exp
    MULT = mybir.AluOpType.mult
    ADD = mybir.AluOpType.add

    nc = bacc.Bacc("TRN2", target_bir_lowering=False, debug=False,
                   num_devices=NCORES)
    xT = nc.dram_tensor("xT", [C, L], BF16, kind="ExternalInput").ap()
    wq = nc.dram_tensor("wq", [C, P], BF16, kind="ExternalInput").ap()
    wk = nc.dram_tensor("wk", [C, P], BF16, kind="ExternalInput").ap()
    wv = nc.dram_tensor("wv", [C, P], BF16, kind="ExternalInput").ap()
    wo = nc.dram_tensor("wo", [P, C], BF16, kind="ExternalInput").ap()
    out0 = nc.dram_tensor("out0", [L, C], BF16, kind="ExternalOutput").ap()
    out1 = nc.dram_tensor("out1", [L, C], BF16, kind="ExternalOutput").ap()
    den01 = nc.dram_tensor("den01", [HEADS_PER_CORE, L], BF16,
                           kind="ExternalOutput").ap()

    with tile.TileContext(nc) as tc:
        import contextlib
        loop_cm = tc.For_i(0, reps, 1) if reps > 1 else contextlib.nullcontext()
        with (
            tc.tile_pool(name="persist", bufs=1) as persist,
            tc.tile_pool(name="xpool", bufs=1) as xpool,
            tc.tile_pool(name="ptp", bufs=5) as ptp,
            tc.tile_pool(name="small", bufs=2) as small,
            tc.tile_pool(name="outp", bufs=4) as outp,
            loop_cm,
        ):
            # preload the exp table set so the first real exp doesn't pay
            # the ~2.7us ACT_TABLE_LOAD mid-pipeline
            warm_in = small.tile([1, 8], F32, tag="warm_in")
            warm_out = small.tile([1, 8], F32, tag="warm_out")
            nc.vector.memset(warm_in, 0.0)
            nc.scalar.activation(warm_out, warm_in, EXP)

            # ---- load inputs ----
            wq_t = persist.tile([P, NCC, P], BF16)
            wk_t = persist.tile([P, NCC, P], BF16)
            wv_t = persist.tile([P, NCC, P], BF16)
            # wo rows per head, both at base partition 0 (matmul operands
            # must share base_partition with the attn lhsT)
            wo_th = [persist.tile([DH, C], BF16, name=f"wo_t{_h}")
                     for _h in range(HEADS_PER_CORE)]
            nc.sync.dma_start(wq_t, wq.rearrange("(k p) m -> p k m", p=P))
            nc.sync.dma_start(wk_t, wk.rearrange("(k p) m -> p k m", p=P))
            nc.sync.dma_start(wv_t, wv.rearrange("(k p) m -> p k m", p=P))
            for _h in range(HEADS_PER_CORE):
                nc.sync.dma_start(wo_th[_h], wo[_h * DH:(_h + 1) * DH, :])

            xt = xpool.tile([P, NCC, L], BF16)
            xTr = xT.rearrange("(k p) n -> p k n", p=P)
            for j in range(8):  # split the 8.4MB load across DMA queues
                sl = slice(j * (L // 8), (j + 1) * (L // 8))
                nc.sync.dma_start(xt[:, :, sl], xTr[:, :, sl])

            qT = persist.tile([P, L], BF16)
            kT = persist.tile([P, L], BF16)
            # per head block: [V_h (64) | ones (1) | zero pad (63)]
            v_store = persist.tile([P, NKT, HEADS_PER_CORE, P], BF16)
            # rows 0:64 = unnormalized attention, row 64 = denominator
            attn = [persist.tile([DH + 1, L], BF16, name=f"attn{_h}")
                    for _h in range(HEADS_PER_CORE)]

            # ---- projections ----
            with tc.tile_pool(name="pj_ps", bufs=2, space="PSUM") as pj_ps:
                drain_idx = 0
                for dst, w_t in ((kT, wk_t), (qT, wq_t)):
                    for j in range(L // 512):
                        ps = pj_ps.tile([P, 512], F32, tag="qk_ps")
                        for c in range(NCC):
                            nc.tensor.matmul(
                                ps, w_t[:, c, :],
                                xt[:, c, j * 512:(j + 1) * 512],
                                start=(c == 0), stop=(c == NCC - 1),
                            )
                        dsl = dst[:, j * 512:(j + 1) * 512]
                        if drain_idx % 2 == 0:
                            nc.scalar.copy(dsl, ps)
                        else:
                            nc.vector.tensor_copy(dsl, ps)
                        drain_idx += 1

                # V: per 128-token tile, [tokens, 128] = xT-chunk.T @ wv
                nc.vector.memset(v_store, 0.0)
                ones_t = small.tile([P, NKT], F32, tag="ones")
                nc.vector.memset(ones_t, 1.0)
                for h in range(HEADS_PER_CORE):
                    nc.vector.tensor_copy(v_store[:, :, h, DH], ones_t)
                for r in range(NKT):
                    ps = pj_ps.tile([P, P], F32, tag="v_ps")
                    for c in range(NCC):
                        nc.tensor.matmul(
                            ps, xt[:, c, r * P:(r + 1) * P], wv_t[:, c, :],
                            start=(c == 0), stop=(c == NCC - 1),
                        )
                    nc.vector.tensor_copy(v_store[:, r, 0, 0:DH], ps[:, 0:DH])
                    nc.scalar.copy(v_store[:, r, 1, 0:DH], ps[:, DH:2 * DH])

            # ---- attention ----
            s_ps_cm = tc.tile_pool(name="s_ps", bufs=3, space="PSUM")
            a_ps_cm = tc.tile_pool(name="a_ps", bufs=2, space="PSUM")
            s_ps = s_ps_cm.__enter__()
            a_ps = a_ps_cm.__enter__()
            for qc in range(NQC):
                qsl = slice(qc * QCHUNK, (qc + 1) * QCHUNK)
                att = [a_ps.tile([P, QCHUNK], F32, tag="att", name=f"att{_h}")
                       for _h in range(HEADS_PER_CORE)]
                pend = []  # (kt, pt) waiting for their AV matmuls
                for kt in range(NKT):
                    sps = s_ps.tile([P, 2 * QCHUNK], F32, tag="spsum")
                    for h in range(HEADS_PER_CORE):
                        hsl = slice(h * DH, (h + 1) * DH)
                        # explicit tile_position packs the two K=64 matmuls
                        # into disjoint row-halves of the PE so they co-run
                        nc.tensor.matmul(
                            sps[:, h * QCHUNK:(h + 1) * QCHUNK],
                            kT[hsl, kt * P:(kt + 1) * P],
                            qT[hsl, qsl],
                            start=True, stop=True,
                            tile_position=(h * DH, 0),
                        )
                    pt = ptp.tile([P, 2 * QCHUNK], BF16, tag="pt")
                    if kt % 2 == 1:
                        nc.vector.tensor_scalar(
                            pt.bitcast(I16), sps, SCH_A, SCH_B, MULT, ADD)
                    else:
                        nc.scalar.activation(pt, sps, EXP)
                    pend.append((kt, pt))
                    if len(pend) > 1:
                        okt, opt = pend.pop(0)
                        for h in range(HEADS_PER_CORE):
                            nc.tensor.matmul(
                                att[h],
                                v_store[:, okt, h, :],
                                opt[:, h * QCHUNK:(h + 1) * QCHUNK],
                                start=(okt == 0), stop=(okt == NKT - 1),
                            )
                for okt, opt in pend:
                    for h in range(HEADS_PER_CORE):
                        nc.tensor.matmul(
                            att[h],
                            v_store[:, okt, h, :],
                            opt[:, h * QCHUNK:(h + 1) * QCHUNK],
                            start=(okt == 0), stop=(okt == NKT - 1),
                        )
                # drain unnormalized attention + denominator row
                nc.scalar.copy(attn[0][:, qsl], att[0][0:DH + 1, :])
                nc.vector.tensor_copy(attn[1][:, qsl], att[1][0:DH + 1, :])
            a_ps_cm.__exit__(None, None, None)
            s_ps_cm.__exit__(None, None, None)

            # ---- output projection (per head, unnormalized) ----
            # all 8 PSUM banks are free here: deep ring keeps the PE dense
            with tc.tile_pool(name="o_ps", bufs=8, space="PSUM") as o_ps:
                for qt in range(L // P):
                    for h, out_h in ((0, out0), (1, out1)):
                        ps = o_ps.tile([P, C], F32, tag="o_ps")
                        nc.tensor.matmul(
                            ps, attn[h][0:DH, qt * P:(qt + 1) * P],
                            wo_th[h], start=True, stop=True)
                        osb = outp.tile([P, C], BF16, tag="osb")
                        if (2 * qt + h) % 2 == 0:
                            nc.scalar.copy(osb, ps)
                        else:
                            nc.vector.tensor_copy(osb, ps)
                        nc.sync.dma_start(out_h[qt * P:(qt + 1) * P, :], osb)
                for h in range(HEADS_PER_CORE):
                    nc.sync.dma_start(den01[h:h + 1, :], attn[h][DH:DH + 1, :])

    nc.compile()
    return nc


def _get_nc(reps=1):
    key = f"nc{reps}"
    if key not in _cached:
        _cached[key] = _build(reps)
    return _cached[key]


def _build_in_maps(inputs):
    x = np.asarray(inputs["x"], dtype=np.float32)
    Wq = np.asarray(inputs["Wq"], dtype=np.float32)
    Wk = np.asarray(inputs["Wk"], dtype=np.float32)
    Wv = np.asarray(inputs["Wv"], dtype=np.float32)
    Wo = np.asarray(inputs["Wo"], dtype=np.float32)

    scale = np.float32(1.0 / np.sqrt(DH))
    in_maps = []
    for core in range(NCORES):
        b = core // CORES_PER_BATCH
        j = core % CORES_PER_BATCH
        csl = slice(j * P, (j + 1) * P)
        bf = ml_dtypes.bfloat16
        in_maps.append({
            "xT": np.ascontiguousarray(x[b].T.astype(bf)),
            "wq": np.ascontiguousarray((Wq[:, csl] * scale).astype(bf)),
            "wk": np.ascontiguousarray(Wk[:, csl].astype(bf)),
            "wv": np.ascontiguousarray(Wv[:, csl].astype(bf)),
            "wo": np.ascontiguousarray(Wo[csl, :].astype(bf)),
        })
    return in_maps


def kernel(x, Wq, Wk, Wv, Wo, bo):
    from concourse import bass_utils

    bo = np.asarray(bo, dtype=np.float32)
    in_maps = _build_in_maps(
        {"x": x, "Wq": Wq, "Wk": Wk, "Wv": Wv, "Wo": Wo})

    res = bass_utils.run_bass_kernel_spmd(
        _get_nc(), in_maps, core_ids=list(range(NCORES)))

    out = np.zeros((B, L, C), dtype=np.float32)
    for core in range(NCORES):
        r = res.results[core]
        den = np.asarray(r["den01"]).astype(np.float32)  # [2, L]
        b = core // CORES_PER_BATCH
        out[b] += np.asarray(r["out0"]).astype(np.float32) / den[0][:, None]
        out[b] += np.asarray(r["out1"]).astype(np.float32) / den[1][:, None]
    out += bo[None, None, :]
    return out
